# revision 7
# baseline (speedup 1.0000x reference)
"""ComplEx forward (BN + fused GEMM + sigmoid) on 8 TRN2 NeuronCores.

Strategy (entity-parallel, per the sharding hint):
  - all_ent_emb [100000, 512] is sharded row-wise: 12500 entities per core,
    zero-padded to 12800 so each core runs 25 uniform chunks of 512.
  - e1/rel/gamma/beta are replicated; every core redundantly computes the two
    BatchNorms and combined = bn0(e1)*(re_rel+im_rel) + bn1(e1)*(re_rel-im_rel)
    in a cheap prologue (d-major layout so BN reduces along the free axis).
  - Each core computes its [1024, 12800] score slab with fp32r matmuls
    (full-rate 4-byte PE mode), applies sigmoid on the scalar engine, and
    DMAs the fp32 result out.  Host concatenates the 8 slabs.

Layout notes:
  - The contraction dim (d=512) must sit on SBUF partitions for both matmul
    operands, so ent chunks are transposed on the PE (fp32 has no DMA
    transpose) and combined is transposed once in the prologue.
  - Output orientation is [b-partitions, entity-free] so the DRAM writes are
    2KB-contiguous per partition row.
"""

import sys

if "/opt/trn_rl_repo" not in sys.path:
    sys.path.insert(0, "/opt/trn_rl_repo")

import numpy as np
from contextlib import ExitStack

import concourse.bass as bass
import concourse.tile as tile
from concourse import bacc, mybir
from concourse.bass_utils import run_bass_kernel_spmd
from concourse.masks import make_identity

P = 128
B = 1024  # batch
D = 512  # embedding dim
KT = D // P  # 4 k-tiles
BT = B // P  # 8 b-tiles
N_TOTAL = 100000
NCORES = 8
N_REAL = N_TOTAL // NCORES  # 12500 real entities per core
NCHUNK = 512
CHUNKS = 25
N_SLAB = NCHUNK * CHUNKS  # 12800 (padded)
BN_EPS = 1e-5

F32 = mybir.dt.float32
F32R = mybir.dt.float32r

_CACHE = {}


def _build():
    nc = bacc.Bacc(None, target_bir_lowering=False)

    e1 = nc.dram_tensor("e1", [B, D], F32, kind="ExternalInput")
    rel = nc.dram_tensor("rel", [B, D], F32, kind="ExternalInput")
    ent = nc.dram_tensor("ent", [N_SLAB, D], F32, kind="ExternalInput")
    # gamma0, beta0, gamma1, beta1 stacked host-side
    gb = nc.dram_tensor("gb", [4, D], F32, kind="ExternalInput")
    out = nc.dram_tensor("out", [B, N_SLAB], F32, kind="ExternalOutput")

    out_pbn = out[:].rearrange("(bt p) n -> p bt n", p=P)

    with tile.TileContext(nc) as tc:
        with ExitStack() as ctx:
            const = ctx.enter_context(tc.tile_pool(name="const", bufs=1))
            big = ctx.enter_context(tc.tile_pool(name="big", bufs=1))
            entbm_p = ctx.enter_context(tc.tile_pool(name="entbm", bufs=3))
            enttp_p = ctx.enter_context(tc.tile_pool(name="enttp", bufs=3))
            outsb_p = ctx.enter_context(tc.tile_pool(name="outsb", bufs=8))
            pst = ctx.enter_context(tc.tile_pool(name="pst", bufs=3, space="PSUM"))
            pso = ctx.enter_context(tc.tile_pool(name="pso", bufs=4, space="PSUM"))

            ident = const.tile([P, P], F32)
            make_identity(nc, ident)
            eps_t = const.tile([P, 1], F32)
            nc.vector.memset(eps_t, BN_EPS)

            # ---------------- prologue: BN + combined, d-major ----------------
            e1_bm = big.tile([P, BT, D], F32, tag="e1_bm")
            rel_bm = big.tile([P, BT, D], F32, tag="rel_bm")
            nc.sync.dma_start(e1_bm[:], e1[:].rearrange("(bt p) d -> p bt d", p=P))
            nc.sync.dma_start(rel_bm[:], rel[:].rearrange("(bt p) d -> p bt d", p=P))

            gb_sb = const.tile([4, D], F32)
            nc.sync.dma_start(gb_sb[:], gb[:])
            gbT = const.tile([P, KT, 4], F32)

            # reT ends up in e1T's buffer, sT in relT's.
            e1T = big.tile([P, KT, B], F32, tag="e1T")  # -> reT -> re*s
            relT = big.tile([P, KT, B], F32, tag="relT")  # -> sT
            imT = big.tile([P, KT, B], F32, tag="imT")
            dT = big.tile([P, KT, B], F32, tag="dT")
            combT = big.tile([P, KT, B], F32R, tag="combT")

            for k in range(KT):
                # gamma/beta columns for this k-tile: transpose [4, 128] -> [128, 4]
                ps_g = pst.tile([P, NCHUNK], F32, tag="pst")
                nc.tensor.transpose(
                    ps_g[:, :4],
                    gb_sb[:4, k * P : (k + 1) * P],
                    ident[:4, :4],
                )
                nc.vector.tensor_copy(gbT[:, k, :], ps_g[:, :4])

                for bt in range(BT):
                    ps_e = pst.tile([P, NCHUNK], F32, tag="pst")
                    nc.tensor.transpose(
                        ps_e[:, :P],
                        e1_bm[:, bt, k * P : (k + 1) * P],
                        ident[:],
                    )
                    nc.vector.tensor_copy(e1T[:, k, bt * P : (bt + 1) * P], ps_e[:, :P])
                    ps_r = pst.tile([P, NCHUNK], F32, tag="pst")
                    nc.tensor.transpose(
                        ps_r[:, :P],
                        rel_bm[:, bt, k * P : (k + 1) * P],
                        ident[:],
                    )
                    nc.vector.tensor_copy(relT[:, k, bt * P : (bt + 1) * P], ps_r[:, :P])

            for k in range(KT):
                # BN stats over the batch (free axis), 2 subgroups of 512
                stats = const.tile([P, 2, 6], F32, tag="bnstats")
                xk = e1T[:, k, :].rearrange("p (s f) -> p s f", s=2)
                nc.vector.bn_stats(stats[:, 0, :], xk[:, 0, :])
                nc.vector.bn_stats(stats[:, 1, :], xk[:, 1, :])
                mv = const.tile([P, 2], F32, tag="bnmv")
                nc.vector.bn_aggr(mv[:], stats[:])
                mean = mv[:, 0:1]
                var = mv[:, 1:2]
                rstd = const.tile([P, 1], F32, tag="rstd")
                nc.scalar.activation(
                    rstd[:], var, mybir.ActivationFunctionType.Sqrt, bias=eps_t[:]
                )
                nc.vector.reciprocal(rstd[:], rstd[:])

                # re = e1 * a0 + b0',  a0 = rstd*gamma0, b0' = beta0 - mean*a0
                a0 = const.tile([P, 1], F32, tag="a0")
                b0p = const.tile([P, 1], F32, tag="b0p")
                a1 = const.tile([P, 1], F32, tag="a1")
                b1p = const.tile([P, 1], F32, tag="b1p")
                nc.vector.tensor_mul(a0[:], rstd[:], gbT[:, k, 0:1])
                nc.vector.tensor_mul(b0p[:], mean, a0[:])
                nc.vector.tensor_tensor(
                    b0p[:], gbT[:, k, 1:2], b0p[:], mybir.AluOpType.subtract
                )
                nc.vector.tensor_mul(a1[:], rstd[:], gbT[:, k, 2:3])
                nc.vector.tensor_mul(b1p[:], mean, a1[:])
                nc.vector.tensor_tensor(
                    b1p[:], gbT[:, k, 3:4], b1p[:], mybir.AluOpType.subtract
                )

                # im first (needs raw e1T), then re in-place over e1T
                nc.vector.tensor_scalar(
                    imT[:, k, :],
                    e1T[:, k, :],
                    scalar1=a1[:],
                    scalar2=b1p[:],
                    op0=mybir.AluOpType.mult,
                    op1=mybir.AluOpType.add,
                )
                nc.vector.tensor_scalar(
                    e1T[:, k, :],
                    e1T[:, k, :],
                    scalar1=a0[:],
                    scalar2=b0p[:],
                    op0=mybir.AluOpType.mult,
                    op1=mybir.AluOpType.add,
                )
                # re_rel = im_rel = rel: s = re_rel + im_rel, d = re_rel - im_rel
                nc.vector.tensor_tensor(
                    dT[:, k, :], relT[:, k, :], relT[:, k, :], mybir.AluOpType.subtract
                )
                nc.vector.tensor_tensor(
                    relT[:, k, :], relT[:, k, :], relT[:, k, :], mybir.AluOpType.add
                )
                # combT = re*s + im*d  (final add writes the fp32r-rounded tile)
                nc.vector.tensor_mul(e1T[:, k, :], e1T[:, k, :], relT[:, k, :])
                nc.vector.tensor_mul(imT[:, k, :], imT[:, k, :], dT[:, k, :])
                nc.vector.tensor_add(combT[:, k, :], e1T[:, k, :], imT[:, k, :])

            # ---------------- main loop over entity chunks ----------------
            ent_pbn = ent[:].rearrange("(c nt p) d -> c p nt d", p=P, nt=NCHUNK // P)
            for ci in range(CHUNKS):
                ent_bm = entbm_p.tile([P, NCHUNK // P, D], F32, tag="ent_bm")
                nc.sync.dma_start(ent_bm[:], ent_pbn[ci])

                entT = enttp_p.tile([P, KT, NCHUNK], F32R, tag="entT")
                for k in range(KT):
                    ps_t = pst.tile([P, NCHUNK], F32, tag="pst")
                    for nt in range(NCHUNK // P):
                        nc.tensor.transpose(
                            ps_t[:, nt * P : (nt + 1) * P],
                            ent_bm[:, nt, k * P : (k + 1) * P],
                            ident[:],
                        )
                    nc.vector.tensor_copy(entT[:, k, :], ps_t[:])

                for bt in range(BT):
                    ps_o = pso.tile([P, NCHUNK], F32, tag="pso")
                    for k in range(KT):
                        nc.tensor.matmul(
                            ps_o[:],
                            combT[:, k, bt * P : (bt + 1) * P],
                            entT[:, k, :],
                            start=(k == 0),
                            stop=(k == KT - 1),
                        )
                    out_sb = outsb_p.tile([P, NCHUNK], F32, tag="out_sb")
                    nc.scalar.activation(
                        out_sb[:], ps_o[:], mybir.ActivationFunctionType.Sigmoid
                    )
                    nc.sync.dma_start(
                        out_pbn[:, bt, ci * NCHUNK : (ci + 1) * NCHUNK], out_sb[:]
                    )

    nc.compile()
    return nc


def _get_nc():
    if "nc" not in _CACHE:
        _CACHE["nc"] = _build()
    return _CACHE["nc"]


def _run(inputs, trace=False, trace_kwargs=None):
    e1 = np.ascontiguousarray(np.asarray(inputs["e1_emb"], dtype=np.float32))
    rel = np.ascontiguousarray(np.asarray(inputs["rel_emb"], dtype=np.float32))
    ent = np.ascontiguousarray(np.asarray(inputs["all_ent_emb"], dtype=np.float32))
    gb = np.ascontiguousarray(
        np.stack(
            [
                np.asarray(inputs["gamma0"], dtype=np.float32),
                np.asarray(inputs["beta0"], dtype=np.float32),
                np.asarray(inputs["gamma1"], dtype=np.float32),
                np.asarray(inputs["beta1"], dtype=np.float32),
            ]
        )
    )

    in_maps = []
    for c in range(NCORES):
        shard = np.zeros((N_SLAB, D), dtype=np.float32)
        shard[:N_REAL] = ent[c * N_REAL : (c + 1) * N_REAL]
        in_maps.append({"e1": e1, "rel": rel, "ent": shard, "gb": gb})

    nc = _get_nc()
    kwargs = {}
    if trace:
        kwargs["trace"] = True
        if trace_kwargs:
            kwargs.update(trace_kwargs)
    res = run_bass_kernel_spmd(nc, in_maps, core_ids=list(range(NCORES)), **kwargs)
    full = np.concatenate(
        [res.results[c]["out"][:, :N_REAL] for c in range(NCORES)], axis=1
    )
    return full, res


def kernel(**inputs):
    full, _ = _run(inputs)
    return full


def _make_sharded(nc, n_cores=NCORES):
    """Replicate run_bass_via_pjrt's multi-core jit so we can time repeated
    executions with device-resident inputs (NTFF profiling is unavailable
    under this axon client)."""
    import jax
    from jax.sharding import Mesh, PartitionSpec
    from jax.experimental.shard_map import shard_map
    from concourse import bass2jax as b2j

    b2j.install_neuronx_cc_hook()

    partition_name = nc.partition_id_tensor.name if nc.partition_id_tensor else None
    in_names, out_names, out_avals = [], [], []
    for alloc in nc.m.functions[0].allocations:
        if not isinstance(alloc, mybir.MemoryLocationSet):
            continue
        name = alloc.memorylocations[0].name
        if alloc.kind == "ExternalInput":
            if name != partition_name:
                in_names.append(name)
        elif alloc.kind == "ExternalOutput":
            out_names.append(name)
            shape = tuple(alloc.tensor_shape)
            dtype = mybir.dt.np(alloc.dtype)
            out_avals.append(jax.core.ShapedArray(shape, dtype))
    n_params = len(in_names)
    n_outs = len(out_avals)
    all_in_names = list(in_names) + list(out_names)
    if partition_name is not None:
        all_in_names.append(partition_name)

    donate = tuple(range(n_params, n_params + n_outs))

    def _body(*args):
        operands = list(args)
        if partition_name is not None:
            operands.append(b2j.partition_id_tensor())
        outs = b2j._bass_exec_p.bind(
            *operands,
            out_avals=tuple(out_avals),
            in_names=tuple(all_in_names),
            out_names=tuple(out_names),
            lowering_input_output_aliases=(),
            sim_require_finite=True,
            sim_require_nnan=True,
            nc=nc,
        )
        return tuple(outs)

    devices = jax.devices()[:n_cores]
    mesh = Mesh(np.asarray(devices), ("core",))
    in_specs = (PartitionSpec("core"),) * (n_params + n_outs)
    out_specs = (PartitionSpec("core"),) * n_outs
    sharded = jax.jit(
        shard_map(
            _body, mesh=mesh, in_specs=in_specs, out_specs=out_specs, check_rep=False
        ),
        donate_argnums=donate,
        keep_unused=True,
    )
    return sharded, in_names, out_names, out_avals


def benchmark(inputs, iters=5):
    """Time the 8-core NEFF execution.  Returns (per-iter wall ns list,
    baseline ns list) where baseline is a trivial 8-core NEFF measuring
    dispatch overhead."""
    import time
    import jax

    e1 = np.ascontiguousarray(np.asarray(inputs["e1_emb"], dtype=np.float32))
    rel = np.ascontiguousarray(np.asarray(inputs["rel_emb"], dtype=np.float32))
    ent = np.ascontiguousarray(np.asarray(inputs["all_ent_emb"], dtype=np.float32))
    gb = np.ascontiguousarray(
        np.stack(
            [
                np.asarray(inputs["gamma0"], dtype=np.float32),
                np.asarray(inputs["beta0"], dtype=np.float32),
                np.asarray(inputs["gamma1"], dtype=np.float32),
                np.asarray(inputs["beta1"], dtype=np.float32),
            ]
        )
    )
    per_core = []
    for c in range(NCORES):
        shard = np.zeros((N_SLAB, D), dtype=np.float32)
        shard[:N_REAL] = ent[c * N_REAL : (c + 1) * N_REAL]
        per_core.append({"e1": e1, "rel": rel, "ent": shard, "gb": gb})

    nc = _get_nc()
    sharded, in_names, out_names, out_avals = _make_sharded(nc)

    concat_in = [
        np.concatenate([per_core[c][nm] for c in range(NCORES)], axis=0)
        for nm in in_names
    ]
    dev_in = [jax.device_put(a) for a in concat_in]
    jax.block_until_ready(dev_in)

    def one_iter():
        zeros = [
            jax.device_put(
                np.zeros((NCORES * av.shape[0], *av.shape[1:]), av.dtype)
            )
            for av in out_avals
        ]
        jax.block_until_ready(zeros)
        t0 = time.perf_counter()
        outs = sharded(*dev_in, *zeros)
        jax.block_until_ready(outs)
        t1 = time.perf_counter()
        for o in outs:
            o.delete()
        return (t1 - t0) * 1e9

    one_iter()  # warmup/compile
    times = [one_iter() for _ in range(iters)]

    # trivial baseline NEFF: 1-element copy per core
    base_times = _baseline_times(iters)
    return times, base_times


def _baseline_times(iters):
    import time
    import jax

    if "base_nc" not in _CACHE:
        nc = bacc.Bacc(None, target_bir_lowering=False)
        x = nc.dram_tensor("x", [1, 128], F32, kind="ExternalInput")
        y = nc.dram_tensor("y", [1, 128], F32, kind="ExternalOutput")
        with tile.TileContext(nc) as tc:
            with ExitStack() as ctx:
                sb = ctx.enter_context(tc.tile_pool(name="sb", bufs=1))
                t = sb.tile([1, 128], F32)
                nc.sync.dma_start(t[:], x[:])
                nc.sync.dma_start(y[:], t[:])
        nc.compile()
        _CACHE["base_nc"] = nc
    nc = _CACHE["base_nc"]
    sharded, in_names, out_names, out_avals = _make_sharded(nc)
    xin = jax.device_put(np.zeros((NCORES * 1, 128), np.float32))

    def one_iter():
        zeros = [
            jax.device_put(np.zeros((NCORES * 1, 128), np.float32))
            for _ in out_avals
        ]
        jax.block_until_ready(zeros)
        t0 = time.perf_counter()
        outs = sharded(xin, *zeros)
        jax.block_until_ready(outs)
        t1 = time.perf_counter()
        for o in outs:
            o.delete()
        return (t1 - t0) * 1e9

    one_iter()
    return [one_iter() for _ in range(iters)]


if __name__ == "__main__":
    rng = np.random.default_rng(0)
    ins = {
        "e1_emb": rng.standard_normal((B, D), dtype=np.float32),
        "rel_emb": rng.standard_normal((B, D), dtype=np.float32),
        "all_ent_emb": rng.standard_normal((N_TOTAL, D), dtype=np.float32),
        "gamma0": np.ones(D, np.float32),
        "beta0": np.zeros(D, np.float32),
        "gamma1": np.ones(D, np.float32),
        "beta1": np.zeros(D, np.float32),
    }
    out = kernel(**ins)
    print("out", out.shape, out.dtype, out.min(), out.max())
